# revision 1
# baseline (speedup 1.0000x reference)
"""Multi-head attention (cosine-similarity scores, q=k=v) on 8 trn2 cores.

Reference computation (per head h, batch b):
    h_bh = sin_b @ Wx_h + bx_h                       # [S, F]
    C    = (h_bh h_bh^T) / (|h_s||h_t|)              # cosine scores, symmetric
    P    = softmax(C, axis=-1)                       # no max-shift needed: |C|<=1
    out_bh = P @ h_bh                                # [S, F]
    out_b  = concat_h(out_bh) @ Wp + bp              # [S, D]

Sharding: tensor-parallel over heads. Each core owns HPC=2 heads, computes the
partial output projection for its heads over the full batch, and the host sums
the 8 partials (+bp).

Per-core kernel layout trick: all score/value matmuls run in the "column"
orientation [t-partition, s-free]. Because C is symmetric, exp(C)[t,s] stored
column-wise is exactly the E[s,t] operand needed for Y^T = h^T E, so no
on-chip transpose of the 2048x2048 score matrix is ever needed. The softmax
denominator comes for free from a ones-column appended to the value stationary
operand (out partition 64 of the Y psum accumulates sum_t E[t,s]).

All matmuls use bfloat16 operands (full PE stream rate, and unlike
fp32r the weight load overlaps the previous matmul), accumulating in
fp32 PSUM.
"""
import numpy as np

import concourse.bacc as bacc
import concourse.tile as tile
import concourse.mybir as mybir
from concourse import bass_isa, masks
from concourse.bass_utils import run_bass_kernel_spmd

B, S, D, H, F = 4, 2048, 1024, 16, 64
NCORES = 8
HPC = H // NCORES          # 2 heads per core
FL = HPC * F               # 128 local feature columns
SCH = 512                  # s-chunk (matmul moving dim)
NCH = S // SCH             # 4
KT = D // 128              # 8 contraction tiles for the input projection
NT0 = S // 128             # 16 t-blocks

DEBUG_DUMPS = False

FP = mybir.dt.float32
BF = mybir.dt.bfloat16
NP_BF = mybir.dt.np(mybir.dt.bfloat16)
AF = mybir.ActivationFunctionType


QH = 2 * SCH               # 1024-wide score/exp blocks
NQH = S // QH              # 2


def _build_nc():
    nc = bacc.Bacc("TRN2", target_bir_lowering=False, debug=False)

    sinT = nc.dram_tensor("sinT", [B, KT, S // QH, 128, QH], BF,
                          kind="ExternalInput")
    wxl = nc.dram_tensor("wxl", [128, KT * FL], BF, kind="ExternalInput")
    bxl = nc.dram_tensor("bxl", [FL, 1], FP, kind="ExternalInput")
    wpl = nc.dram_tensor("wpl", [FL, D], BF, kind="ExternalInput")
    outp = nc.dram_tensor("outp", [B, S, D], BF, kind="ExternalOutput")

    with tile.TileContext(nc) as tc:
        with (
            tc.tile_pool(name="const", bufs=1) as constp,
            tc.tile_pool(name="wpool", bufs=1) as wpool,
            tc.tile_pool(name="sin", bufs=16) as sinp,
            tc.tile_pool(name="pa", bufs=1) as pa,
            tc.tile_pool(name="pb", bufs=2) as pb,
            tc.tile_pool(name="epool", bufs=3) as epool,
            tc.tile_pool(name="tail", bufs=2) as tailp,
            tc.tile_pool(name="opool", bufs=3) as opool,
            # 8 PSUM banks, phase-disjoint:
            #   ps_b  2 x [128,1024] = 4 banks (score blocks, phase B)
            #   ps_y  1 x [65,1024]  = 2 banks (Y accumulator, phase B)
            #   ps_ac 1 x [128,512]  = 1 bank  (projection A + out-proj C)
            #   ps_sm 1 x 1 bank               (transposes + norm rows, A)
            tc.tile_pool(name="ps_b", bufs=2, space="PSUM") as ps_b,
            tc.tile_pool(name="ps_y", bufs=1, space="PSUM") as ps_y,
            tc.tile_pool(name="ps_ac", bufs=1, space="PSUM") as ps_ac,
            tc.tile_pool(name="ps_sm", bufs=1, space="PSUM") as ps_sm,
        ):
            # ---- constants / weights ----
            ident = constp.tile([128, 128], FP, tag="ident")
            masks.make_identity(nc, ident[:])

            ones2_f = constp.tile([128, 2], FP, tag="ones2f")
            nc.vector.memset(ones2_f[:], 0.0)
            nc.vector.memset(ones2_f[0:64, 0:1], 1.0)
            nc.vector.memset(ones2_f[64:128, 1:2], 1.0)
            ones2 = constp.tile([128, 2], BF, tag="ones2")
            nc.vector.tensor_copy(ones2[:], ones2_f[:])

            ones16_f = constp.tile([128, NT0], FP, tag="ones16f")
            nc.vector.memset(ones16_f[:], 1.0)

            wx_t = wpool.tile([128, KT * FL], BF, tag="wx")
            nc.sync.dma_start(wx_t[:], wxl.ap())
            bx_t = wpool.tile([FL, 1], FP, tag="bx")
            nc.sync.dma_start(bx_t[:], bxl.ap())
            wp_t = wpool.tile([FL, D], BF, tag="wp")
            nc.sync.dma_start(wp_t[:], wpl.ap())

            # per-batch persistent tiles, created by the A-parts
            state = {}

            def a_parts(b):
                """Emitters for phase A of batch b (projection/norms/aug)."""
                st = {}
                state[b] = st

                def mk_tiles():
                    st["hT"] = pa.tile([128, S], FP, tag="hT",
                                       name=f"hT_{b}")
                    st["sqt"] = pa.tile([128, S], BF, tag="sq",
                                        name=f"sq_{b}")
                    st["norms"] = [
                        pa.tile([1, S], FP, tag=f"norm{h}", name=f"norm{h}_{b}")
                        for h in range(HPC)
                    ]

                sints = {}

                def a1d(c):
                    # c indexes a 1024-wide s-block: one [128,1024] sin DMA
                    # per k-tile (fewer, bigger transfers)
                    if c == 0:
                        mk_tiles()
                    sints[c] = []
                    for k in range(KT):
                        sint = sinp.tile([128, QH], BF, tag="sin",
                                         name=f"sin_{b}_{c}_{k}")
                        nc.sync.dma_start(sint[:], sinT.ap()[b, k, c])
                        sints[c].append(sint)

                def a1q(c, half, q):
                    # one quarter (4 k-tiles) of a 512-wide projection psum
                    # accumulation; emitted fine-grained so the PE bursts
                    # interleaved into phase B stay under the exp
                    # double-buffer's slack
                    cs = slice(c * QH + half * SCH,
                               c * QH + (half + 1) * SCH)
                    hs = slice(half * SCH, (half + 1) * SCH)
                    if q == 0:
                        st[f"pshT{half}"] = ps_ac.tile(
                            [128, SCH], FP, tag="ac",
                            name=f"pshT_{b}_{c}_{half}")
                    pshT = st[f"pshT{half}"]
                    for k in range(q * KT // 2, (q + 1) * KT // 2):
                        nc.tensor.matmul(
                            pshT[:], wx_t[:, k * FL:(k + 1) * FL],
                            sints[c][k][:, hs],
                            start=(k == 0), stop=(k == KT - 1),
                        )
                    if q == 1:
                        nc.vector.tensor_scalar_add(st["hT"][:, cs], pshT[:],
                                                    bx_t[:])
                        nc.vector.tensor_mul(st["sqt"][:, cs], st["hT"][:, cs],
                                             st["hT"][:, cs])

                def a1n(g):
                    # norm^2 partition reductions; rows copied to SBUF (DVE)
                    for c in range(g * 2, g * 2 + 2):
                        cs = slice(c * SCH, (c + 1) * SCH)
                        for h in range(HPC):
                            psn = ps_sm.tile([1, SCH], FP, tag="sm",
                                             name=f"psn_{b}_{c}_{h}")
                            nc.tensor.matmul(psn[:], ones2[:, h:h + 1],
                                             st["sqt"][:, cs],
                                             start=True, stop=True)
                            nc.vector.tensor_copy(st["norms"][h][:, cs],
                                                  psn[:])

                def a2():
                    # rnb <- broadcast(norm2), then ONE full-width ACT sqrt
                    # (one table-swap pair per batch instead of two) and a
                    # fast reciprocal
                    rnb = pa.tile([128, S], FP, tag="rnb", name=f"rnb_{b}")
                    rnb1 = pa.tile([F, S], FP, tag="rnb1", name=f"rnb1_{b}")
                    st["hTn"] = pb.tile([128, S], BF, tag="hTn",
                                        name=f"hTn_{b}")
                    nc.gpsimd.partition_broadcast(rnb[0:F, :],
                                                  st["norms"][0][:])
                    nc.gpsimd.partition_broadcast(rnb1[:], st["norms"][1][:])
                    nc.vector.tensor_copy(rnb[F:2 * F, :], rnb1[:])
                    nc.scalar.sqrt(rnb[:], rnb[:])
                    nc.vector.reciprocal_approx_fast(rnb[:], rnb[:])
                    nc.vector.tensor_mul(st["hTn"][:], st["hT"][:], rnb[:])
                    st["augs"] = [
                        pb.tile([128, NT0 * (F + 1)], BF, tag=f"aug{h}",
                                name=f"aug{h}_{b}")
                        for h in range(HPC)
                    ]

                def a3(q):
                    # 4 transposes per call, emitted fine-grained as fillers
                    for t0 in range(q * 4, q * 4 + 4):
                        pst = ps_sm.tile([128, 128], FP, tag="sm",
                                         name=f"pst_{b}_{t0}")
                        nc.tensor.transpose(
                            pst[:], st["hT"][:, t0 * 128:(t0 + 1) * 128],
                            ident[:]
                        )
                        for h in range(HPC):
                            nc.vector.tensor_copy(
                                st["augs"][h][:,
                                              t0 * (F + 1):t0 * (F + 1) + F],
                                pst[:, h * F:(h + 1) * F],
                            )

                def a4():
                    for h in range(HPC):
                        ones_col = st["augs"][h][:].rearrange(
                            "p (i c) -> p i c", c=F + 1
                        )[:, :, F:F + 1]
                        nc.vector.tensor_copy(ones_col, ones16_f[:])
                    st["outT"] = pb.tile([128, S], BF, tag="outT",
                                         name=f"outT_{b}")

                return (
                    [lambda: a1d(0)]
                    + [lambda h_=h_, q=q: a1q(0, h_, q)
                       for h_ in range(2) for q in range(2)]
                    + [lambda: a1d(1)]
                    + [lambda h_=h_, q=q: a1q(1, h_, q)
                       for h_ in range(2) for q in range(2)]
                    + [lambda: a1n(0), lambda: a1n(1), a2]
                    + [lambda q=q: a3(q) for q in range(3)]
                    + [lambda: (a3(3), a4())]
                )

            def b_part(b, h, qh, fillers=(), end_fillers=()):
                """One quarter of phase B: head h, s-halfblock qh.

                Fillers (A/C-phase emitters) are interleaved after every
                other t-block so their PE bursts stay within the slack the
                exp double-buffer gives, instead of starving the ACT engine
                with one long burst between quarters.
                """
                st = state[b]
                hr = slice(h * F, (h + 1) * F)
                aug = st["augs"][h]
                hTn = st["hTn"]
                qs = slice(qh * QH, (qh + 1) * QH)
                fillers = list(fillers)
                psy = ps_y.tile([F + 1, QH], FP, tag="y",
                                name=f"psy_{b}_{h}_{qh}")
                for t0 in range(NT0):
                    psc = ps_b.tile([128, QH], FP, tag="b",
                                    name=f"psc_{b}_{h}_{qh}_{t0}")
                    et = epool.tile([128, QH], BF, tag="E",
                                    name=f"E_{b}_{h}_{qh}_{t0}")
                    ts0 = slice(t0 * 128, (t0 + 1) * 128)
                    for n in range(QH // SCH):
                        ns = slice(n * SCH, (n + 1) * SCH)
                        nc.tensor.matmul(
                            psc[:, ns], hTn[hr, ts0],
                            hTn[hr, qh * QH + n * SCH:qh * QH + (n + 1) * SCH],
                            start=True, stop=True,
                        )
                    nc.scalar.activation(et[:], psc[:], AF.Exp)
                    for n in range(QH // SCH):
                        ns = slice(n * SCH, (n + 1) * SCH)
                        nc.tensor.matmul(
                            psy[:, ns],
                            aug[:, t0 * (F + 1):(t0 + 1) * (F + 1)],
                            et[:, ns],
                            start=(t0 == 0), stop=(t0 == NT0 - 1),
                        )
                    if t0 % 2 == 1 and fillers:
                        fillers.pop(0)()
                # tail: copy Y out of PSUM fast (frees the accumulator for
                # the next quarter), then divide by the d row.
                # (reciprocal_approx_fast misreads inputs whose AP starts at
                # partition 64 on HW, and PSUM reads must start on an aligned
                # partition, so stage the d row to partition 0 first.)
                ysb = tailp.tile([F, QH], FP, tag="ysb",
                                 name=f"ysb_{b}_{h}_{qh}")
                nc.vector.tensor_copy(ysb[:], psy[0:F, :])
                rdsrc = tailp.tile([1, QH], FP, tag="rdsrc",
                                   name=f"rdsrc_{b}_{h}_{qh}")
                nc.vector.tensor_copy(rdsrc[:], psy[F:F + 1, :])
                rd = tailp.tile([1, QH], FP, tag="rd", name=f"rd_{b}_{h}_{qh}")
                nc.vector.reciprocal_approx_fast(rd[:], rdsrc[:])
                rdb = tailp.tile([F, QH], FP, tag="rdb",
                                 name=f"rdb_{b}_{h}_{qh}")
                nc.gpsimd.partition_broadcast(rdb[:], rd[:])
                nc.vector.tensor_mul(st["outT"][hr, qs], ysb[:], rdb[:])
                for part in fillers:
                    part()
                # ACT-heavy fillers (the sqrt + its table swaps) go at the
                # quarter boundary so they never stall a running exp stream
                for part in end_fillers:
                    part()

            def c_parts(b):
                st = state[b]

                def c1(sb, scalar_copy=False):
                    ss = slice(sb * 128, (sb + 1) * 128)
                    ot = opool.tile([128, D], BF, tag="osb",
                                    name=f"ot_{b}_{sb}")
                    for n in range(D // 512):
                        # alternate psum banks so the two matmuls don't
                        # serialize on the drain of the first
                        pool = ps_ac if n == 0 else ps_sm
                        psp = pool.tile([128, 512], FP, tag="ac" if n == 0
                                        else "sm", name=f"psp_{b}_{sb}_{n}")
                        nc.tensor.matmul(
                            psp[:],
                            st["outT"][:, ss],
                            wp_t[:, n * 512:(n + 1) * 512],
                            start=True, stop=True,
                        )
                        # the post-B tail has an idle ACT engine; splitting
                        # the PSUM drain across both engines shortens it
                        eng = nc.scalar if (scalar_copy and n == 0) else None
                        if eng is not None:
                            eng.copy(ot[:, n * 512:(n + 1) * 512], psp[:])
                        else:
                            nc.vector.tensor_copy(
                                ot[:, n * 512:(n + 1) * 512], psp[:])
                    nc.sync.dma_start(outp.ap()[b, ss, :], ot[:])

                return [
                    lambda sb=sb: c1(sb, scalar_copy=(b == B - 1 and sb >= 8))
                    for sb in range(S // 128)
                ]

            # ---- software-pipelined emission ----
            # B-parts of batch b interleave with: A-parts of b+1, the high
            # half of C(b-1), and the low half of C(b) (whose outT slices
            # complete after the second B-part).
            cl = {}
            for part in a_parts(0):
                part()
            for b in range(B):
                cl[b] = c_parts(b)
                ap = a_parts(b + 1) if b + 1 < B else []
                cprev = cl[b - 1][8:16] if b >= 1 else []
                ccur = cl[b][0:8]
                plan = [
                    ((0, 0), ap[0:5] + cprev[0:4], []),
                    ((1, 0), ap[5:10] + cprev[4:8], ap[10:13]),
                    ((0, 1), ccur[0:4], []),
                    ((1, 1), ap[13:17] + ccur[4:8], []),
                ]
                for (h, qh), fillers, endf in plan:
                    b_part(b, h, qh, fillers, endf)
            for part in cl[B - 1][8:16]:
                part()

    nc.compile()
    return nc

_NC_CACHE = []


def _get_nc():
    if not _NC_CACHE:
        _NC_CACHE.append(_build_nc())
    return _NC_CACHE[0]


def make_in_maps(sin, Wx, bx, Wp):
    """Host-side sharding: per-core input dicts."""
    # [B, D, S] -> contiguous tiles [B, KT, S//QH, 128, QH] so each sin DMA
    # is one 512KB contiguous read
    QH_ = 2 * SCH
    sinT = np.transpose(sin, (0, 2, 1)).reshape(B, KT, 128, S // QH_, QH_)
    sinT = np.ascontiguousarray(
        np.transpose(sinT, (0, 1, 3, 2, 4)).astype(NP_BF)
    )
    in_maps = []
    for c in range(NCORES):
        hs = slice(c * HPC, (c + 1) * HPC)
        # [D, FL] stacked head projections -> [128, KT*FL] k-tile-major
        wxl = np.concatenate([Wx[h] for h in range(c * HPC, (c + 1) * HPC)],
                             axis=1)
        wxl = np.ascontiguousarray(
            wxl.reshape(KT, 128, FL).transpose(1, 0, 2).reshape(128, KT * FL)
        ).astype(NP_BF)
        bxl = np.ascontiguousarray(bx[hs].reshape(FL, 1))
        wpl = np.ascontiguousarray(Wp[c * FL:(c + 1) * FL, :]).astype(NP_BF)
        in_maps.append({"sinT": sinT, "wxl": wxl, "bxl": bxl, "wpl": wpl})
    return in_maps


def make_runner(sin, Wx, bx, Wp):
    """Build a repeat-callable single-execution runner with device-resident
    inputs.

    Outputs are fed back as the donated output buffers, so each call is
    dispatch + device execution only (no host transfers). Returns
    (run_once, block) where run_once() dispatches one execution
    asynchronously and block() waits for all dispatched work.
    """
    import jax
    from concourse import bass2jax as b2j
    from concourse import mybir as _mb

    nc = _get_nc()
    b2j.install_neuronx_cc_hook()
    in_maps = make_in_maps(
        np.asarray(sin, np.float32), np.asarray(Wx, np.float32),
        np.asarray(bx, np.float32), np.asarray(Wp, np.float32),
    )

    in_names, out_names, out_avals, zero_outs = [], [], [], []
    for alloc in nc.m.functions[0].allocations:
        if not isinstance(alloc, _mb.MemoryLocationSet):
            continue
        name = alloc.memorylocations[0].name
        if alloc.kind == "ExternalInput":
            if nc.partition_id_tensor is None or name != nc.partition_id_tensor.name:
                in_names.append(name)
        elif alloc.kind == "ExternalOutput":
            out_names.append(name)
            shape = tuple(alloc.tensor_shape)
            dtype = _mb.dt.np(alloc.dtype)
            out_avals.append(jax.core.ShapedArray(shape, dtype))
            zero_outs.append(np.zeros(shape, dtype))
    n_params = len(in_names)
    n_outs = len(out_avals)
    all_names = in_names + out_names
    donate = tuple(range(n_params, n_params + n_outs))

    pid_name = nc.partition_id_tensor.name if nc.partition_id_tensor else None
    body_names = all_names + ([pid_name] if pid_name else [])

    def _exec_once(ins_, outs_):
        operands = list(ins_) + list(outs_)
        if pid_name:
            operands.append(b2j.partition_id_tensor())
        outs = b2j._bass_exec_p.bind(
            *operands,
            out_avals=tuple(out_avals),
            in_names=tuple(body_names),
            out_names=tuple(out_names),
            lowering_input_output_aliases=(),
            sim_require_finite=True,
            sim_require_nnan=True,
            nc=nc,
        )
        return tuple(outs)

    def _body(*args):
        return _exec_once(args[:n_params], args[n_params:])

    devices = jax.devices()[:NCORES]
    mesh = b2j.Mesh(np.asarray(devices), ("core",))
    in_specs = (b2j.PartitionSpec("core"),) * (n_params + n_outs)
    out_specs = (b2j.PartitionSpec("core"),) * n_outs
    sharded = jax.jit(
        b2j.shard_map(_body, mesh=mesh, in_specs=in_specs,
                      out_specs=out_specs, check_rep=False),
        donate_argnums=donate, keep_unused=True,
    )
    sharding = jax.sharding.NamedSharding(mesh, b2j.PartitionSpec("core"))
    concat_in = [
        jax.device_put(
            np.concatenate([np.asarray(in_maps[c][nm]) for c in range(NCORES)],
                           axis=0),
            sharding,
        )
        for nm in in_names
    ]
    outs = [
        jax.device_put(np.zeros((NCORES * z.shape[0], *z.shape[1:]), z.dtype),
                       sharding)
        for z in zero_outs
    ]
    jax.block_until_ready(concat_in)

    state = {"outs": outs}

    def run_once():
        state["outs"] = sharded(*concat_in, *state["outs"])

    def block():
        jax.block_until_ready(state["outs"])

    return run_once, block


def benchmark(sin, Wx, bx, Wp, iters=10, loop_n=1, runner=None):
    """Timed loop of the compiled executable; returns per-exec ns."""
    import time as _time

    run_once, block = runner or make_runner(sin, Wx, bx, Wp)
    times = []
    for _ in range(iters):
        t0 = _time.perf_counter()
        for _k in range(loop_n):
            run_once()
        block()
        times.append((_time.perf_counter() - t0) * 1e9 / loop_n)
    return times


def kernel(sin, Wx, bx, Wp, bp, _trace=False):
    sin = np.asarray(sin, dtype=np.float32)
    Wx = np.asarray(Wx, dtype=np.float32)
    bx = np.asarray(bx, dtype=np.float32)
    Wp = np.asarray(Wp, dtype=np.float32)
    bp = np.asarray(bp, dtype=np.float32)

    nc = _get_nc()
    in_maps = make_in_maps(sin, Wx, bx, Wp)
    res = run_bass_kernel_spmd(nc, in_maps, list(range(NCORES)), trace=_trace)
    out = np.sum(np.stack([np.asarray(r["outp"], np.float32)
                       for r in res.results]), axis=0) + bp
    if _trace:
        kernel.last_results = res
    return out.astype(np.float32)



# revision 7
# speedup vs baseline: 1.0430x; 1.0430x over previous
"""Multi-head attention (cosine-similarity scores, q=k=v) on 8 trn2 cores.

Reference computation (per head h, batch b):
    h_bh = sin_b @ Wx_h + bx_h                       # [S, F]
    C    = (h_bh h_bh^T) / (|h_s||h_t|)              # cosine scores, symmetric
    P    = softmax(C, axis=-1)                       # no max-shift needed: |C|<=1
    out_bh = P @ h_bh                                # [S, F]
    out_b  = concat_h(out_bh) @ Wp + bp              # [S, D]

Sharding: tensor-parallel over heads. Each core owns HPC=2 heads, computes the
partial output projection for its heads over the full batch, and the host sums
the 8 partials (+bp).

Layout: all score/value matmuls run column-wise [t-partition, s-free]; C's
symmetry makes exp(C) stored column-wise exactly the E[s,t] operand for
Y^T = h^T E.  The two heads live on partitions 0-63 / 64-127, so their K=64
score matmuls run CONCURRENTLY in the PE array (row-group tiling) and one
[128,1024] exp covers both heads (ACT is the critical engine; its per-call
overhead is amortized at the max width 4 PSUM banks allow).  The softmax
denominator rides as a ones-column in the Y stationary (psum row 64).  rsqrt
of the norms is exp(-0.5*ln(x)) so the whole kernel uses ONE ACT table set
(natural_log_exp_and_others) - no table swaps.  Norm reciprocals are
partition-broadcast with tiny K=1 matmuls (gpsimd broadcast can only read
partition 0; PE does it for free in the slack).
"""
import numpy as np

import concourse.bacc as bacc
import concourse.tile as tile
import concourse.mybir as mybir
from concourse import bass_isa, masks
from concourse.bass_utils import run_bass_kernel_spmd

B, S, D, H, F = 4, 2048, 1024, 16, 64
NCORES = 8
HPC = H // NCORES          # 2 heads per core
FL = HPC * F               # 128 local feature columns
SCH = 512                  # s-chunk width (one psy bank)
NCH = S // SCH             # 4 chunks per batch
KT = D // 128              # 8 contraction tiles for the input projection
NT0 = S // 128             # 16 t-blocks
QH = 1024                  # sin DMA block width
AUGW = NT0 * (F + 1)       # 1040 aug columns per head

FP = mybir.dt.float32
BF = mybir.dt.bfloat16
NP_BF = mybir.dt.np(mybir.dt.bfloat16)
AF = mybir.ActivationFunctionType


def _build_nc():
    nc = bacc.Bacc("TRN2", target_bir_lowering=False, debug=False)

    sinT = nc.dram_tensor("sinT", [B, KT, S // QH, 128, QH], BF,
                          kind="ExternalInput")
    wxl = nc.dram_tensor("wxl", [128, KT * FL], BF, kind="ExternalInput")
    bxl = nc.dram_tensor("bxl", [FL, 1], FP, kind="ExternalInput")
    wpl = nc.dram_tensor("wpl", [FL, D], BF, kind="ExternalInput")
    outp = nc.dram_tensor("outp", [B, S, D], BF, kind="ExternalOutput")

    with tile.TileContext(nc) as tc:
        with (
            tc.tile_pool(name="const", bufs=1) as constp,
            tc.tile_pool(name="wpool", bufs=1) as wpool,
            tc.tile_pool(name="sin", bufs=16) as sinp,
            tc.tile_pool(name="pa", bufs=1) as pa,
            tc.tile_pool(name="pb", bufs=2) as pb,
            tc.tile_pool(name="epool", bufs=3) as epool,
            tc.tile_pool(name="tailp", bufs=2) as tailp,
            tc.tile_pool(name="opool", bufs=3) as opool,
            # 8 PSUM banks:
            #   ps_c  2 x [128,1024] = 4 banks (paired score blocks)
            #   ps_y  2 x [65,512]   = 2 banks (per-head Y accumulators)
            #   ps_a  1 x [128,512]  = 1 bank  (proj accum / psn1 / rnb / outproj)
            #   ps_s  1 x [128,512]  = 1 bank  (psn0 / rnb / transposes / outproj)
            tc.tile_pool(name="ps_c", bufs=2, space="PSUM") as ps_c,
            tc.tile_pool(name="ps_y", bufs=1, space="PSUM") as ps_y,
            tc.tile_pool(name="ps_a", bufs=1, space="PSUM") as ps_a,
            tc.tile_pool(name="ps_s", bufs=1, space="PSUM") as ps_s,
        ):
            # ---- constants / weights ----
            ident = constp.tile([128, 128], FP, tag="ident")
            masks.make_identity(nc, ident[:])

            # ones2[:, h] = 1 on partitions h*64..h*64+63 (norm reductions)
            ones2_f = constp.tile([128, 2], FP, tag="ones2f")
            nc.vector.memset(ones2_f[:], 0.0)
            nc.vector.memset(ones2_f[0:64, 0:1], 1.0)
            nc.vector.memset(ones2_f[64:128, 1:2], 1.0)
            ones2 = constp.tile([128, 2], BF, tag="ones2")
            nc.vector.tensor_copy(ones2[:], ones2_f[:])

            # all-ones [128,64] bf16: K=1 broadcast-matmul stationary
            onesb_f = constp.tile([128, 64], FP, tag="onesbf")
            nc.vector.memset(onesb_f[:], 1.0)
            onesb = constp.tile([128, 64], BF, tag="onesb")
            nc.vector.tensor_copy(onesb[:], onesb_f[:])

            ones16_f = constp.tile([128, NT0], FP, tag="ones16f")
            nc.vector.memset(ones16_f[:], 1.0)

            wx_t = wpool.tile([128, KT * FL], BF, tag="wx")
            nc.sync.dma_start(wx_t[:], wxl.ap())
            bx_t = wpool.tile([FL, 1], FP, tag="bx")
            nc.sync.dma_start(bx_t[:], bxl.ap())
            wp_t = wpool.tile([FL, D], BF, tag="wp")
            nc.sync.dma_start(wp_t[:], wpl.ap())

            state = {}

            def a_parts(b):
                """Emitters for phase A of batch b: projection, norms,
                normalization, aug build.  Returned as a list of parts to be
                interleaved into the previous batch's attention chunks."""
                st = {}
                state[b] = st
                sints = {}

                def mk():
                    st["hT"] = pa.tile([128, S], FP, tag="hT", name=f"hT_{b}")
                    st["sqt"] = pa.tile([128, S], BF, tag="sq", name=f"sq_{b}")
                    st["hTn"] = pb.tile([128, S], BF, tag="hTn",
                                        name=f"hTn_{b}")
                    st["aug"] = pb.tile([128, HPC * AUGW], BF, tag="aug",
                                        name=f"aug_{b}")
                    st["outT"] = pb.tile([128, S], BF, tag="outT",
                                         name=f"outT_{b}")
                    st["ysb"] = [
                        pb.tile([F + 1, S], FP, tag=f"ysb{h}",
                                name=f"ysb{h}_{b}")
                        for h in range(HPC)
                    ]
                    st["rn"] = [
                        pa.tile([128, SCH], BF, tag=f"rn{h}",
                                name=f"rn{h}_{b}")
                        for h in range(HPC)
                    ]

                def a1d(c2):
                    if c2 == 0:
                        mk()
                    sints[c2] = []
                    for k in range(KT):
                        sint = sinp.tile([128, QH], BF, tag="sin",
                                         name=f"sin_{b}_{c2}_{k}")
                        nc.sync.dma_start(sint[:], sinT.ap()[b, k, c2])
                        sints[c2].append(sint)

                def a1q(half, q):
                    # one half (4 k-tiles) of a 512-wide projection accum
                    cs = slice(half * SCH, (half + 1) * SCH)
                    hs = slice((half % 2) * SCH, (half % 2 + 1) * SCH)
                    if q == 0:
                        st["pshT"] = ps_a.tile([128, SCH], FP, tag="a",
                                               name=f"pshT_{b}_{half}")
                    pshT = st["pshT"]
                    for k in range(q * KT // 2, (q + 1) * KT // 2):
                        nc.tensor.matmul(
                            pshT[:], wx_t[:, k * FL:(k + 1) * FL],
                            sints[half // 2][k][:, hs],
                            start=(k == 0), stop=(k == KT - 1),
                        )
                    if q == 1:
                        nc.vector.tensor_scalar_add(st["hT"][:, cs], pshT[:],
                                                    bx_t[:])
                        nc.vector.tensor_mul(st["sqt"][:, cs], st["hT"][:, cs],
                                             st["hT"][:, cs])

                def norms(h):
                    # norm^2 rows for head h -> psum partitions {0,32,64,96}
                    # (chunk c at row c*32), then rnorm = exp(-0.5*ln(x)):
                    # same ACT table set as the attention exp, no table swaps.
                    # Compact: the psum slot is created and consumed within
                    # this one part (the 1-buf pools rotate with outproj).
                    pool = ps_s if h == 0 else ps_a
                    psn = pool.tile([128, SCH], FP, tag="s" if h == 0
                                    else "a", name=f"psn{h}_{b}")
                    for c in range(NCH):
                        cs = slice(c * SCH, (c + 1) * SCH)
                        nc.tensor.matmul(psn[c * 32:c * 32 + 1, :],
                                         ones2[:, h:h + 1], st["sqt"][:, cs],
                                         start=True, stop=True,
                                         tile_position=(0, c * 32))
                    lnt = pa.tile([128, SCH], FP, tag="lnt",
                                  name=f"lnt_{b}_{h}")
                    nc.scalar.activation(lnt[:], psn[:], AF.Ln)
                    nc.scalar.activation(st["rn"][h][:], lnt[:], AF.Exp,
                                         scale=-0.5)

                def rnb(c):
                    # broadcast rnorm rows to [128,512] via K=1 matmuls,
                    # then one mul makes the normalized hTn chunk
                    pool = ps_s if c % 2 == 0 else ps_a
                    cs = slice(c * SCH, (c + 1) * SCH)
                    prn = pool.tile([128, SCH], FP, tag="s" if c % 2 == 0
                                    else "a", name=f"prn_{b}_{c}")
                    r = c * 32
                    nc.tensor.matmul(prn[0:64, :], onesb[r:r + 1, :],
                                     st["rn"][0][r:r + 1, :],
                                     start=True, stop=True,
                                     tile_position=(r, 0))
                    nc.tensor.matmul(prn[64:128, :], onesb[r:r + 1, :],
                                     st["rn"][1][r:r + 1, :],
                                     start=True, stop=True,
                                     tile_position=(r, 64))
                    nc.vector.tensor_mul(st["hTn"][:, cs], st["hT"][:, cs],
                                         prn[:])

                def a3(q):
                    # 4 transposes: hT [f,t] -> aug [t,f] for both heads
                    for t0 in range(q * 4, q * 4 + 4):
                        pool = ps_s if t0 % 2 == 0 else ps_a
                        pst = pool.tile([128, 128], FP, tag="s" if t0 % 2 == 0
                                        else "a", name=f"pst_{b}_{t0}")
                        nc.tensor.transpose(
                            pst[:], st["hT"][:, t0 * 128:(t0 + 1) * 128],
                            ident[:]
                        )
                        dst = st["aug"][:].rearrange(
                            "p (h t f) -> p h t f", h=HPC, f=F + 1
                        )[:, :, t0, 0:F]
                        src = pst[:].rearrange("p (h f) -> p h f", h=HPC)
                        nc.vector.tensor_copy(dst, src)

                def a4():
                    for h in range(HPC):
                        ones_col = st["aug"][:].rearrange(
                            "p (h t f) -> p h t f", h=HPC, f=F + 1
                        )[:, h, :, F:F + 1]
                        nc.vector.tensor_copy(ones_col, ones16_f[:])

                return (
                    [lambda: a1d(0),
                     lambda: a1q(0, 0), lambda: a1q(0, 1),
                     lambda: a1q(1, 0), lambda: a1q(1, 1),
                     lambda: a1d(1),
                     lambda: a1q(2, 0), lambda: a1q(2, 1),
                     lambda: a1q(3, 0), lambda: a1q(3, 1),
                     lambda: norms(0), lambda: norms(1)]
                    + [lambda c=c: rnb(c) for c in range(NCH)]
                    + [lambda q=q: a3(q) for q in range(4)]
                    + [a4]
                )

            def b_chunk(b, c, fillers=(), end_fillers=()):
                """One 512-wide s-chunk: both heads, all 16 t-blocks."""
                st = state[b]
                cs = slice(c * SCH, (c + 1) * SCH)
                fillers = list(fillers)
                psy = [
                    ps_y.tile([F + 1, SCH], FP, tag=f"y{h}",
                              name=f"psy{h}_{b}_{c}")
                    for h in range(HPC)
                ]
                aug4 = st["aug"][:].rearrange("p (h t f) -> p h t f",
                                              h=HPC, f=F + 1)
                for t0 in range(NT0):
                    ts0 = slice(t0 * 128, (t0 + 1) * 128)
                    psc = ps_c.tile([128, 2 * SCH], FP, tag="c",
                                    name=f"psc_{b}_{c}_{t0}")
                    for h in range(HPC):
                        hs = slice(h * F, (h + 1) * F)
                        nc.tensor.matmul(
                            psc[:, h * SCH:(h + 1) * SCH],
                            st["hTn"][hs, ts0], st["hTn"][hs, cs],
                            start=True, stop=True,
                        )
                    et = epool.tile([128, 2 * SCH], BF, tag="E",
                                    name=f"E_{b}_{c}_{t0}")
                    nc.scalar.activation(et[:], psc[:], AF.Exp)
                    for h in range(HPC):
                        nc.tensor.matmul(
                            psy[h][:], aug4[:, h, t0, :],
                            et[:, h * SCH:(h + 1) * SCH],
                            start=(t0 == 0), stop=(t0 == NT0 - 1),
                        )
                    if fillers:
                        fillers.pop(0)()
                # release the psy banks quickly into the per-head accums
                for h in range(HPC):
                    nc.vector.tensor_copy(st["ysb"][h][:, cs], psy[h][:])
                for part in fillers:
                    part()
                for part in end_fillers:
                    part()

            def tail(b, h, half):
                """Divide accumulated Y by the softmax denominators."""
                st = state[b]
                hs2 = slice(half * QH, (half + 1) * QH)
                rdsrc = tailp.tile([1, QH], FP, tag="rdsrc",
                                   name=f"rdsrc_{b}_{h}_{half}")
                nc.vector.tensor_copy(rdsrc[:], st["ysb"][h][F:F + 1, hs2])
                rd = tailp.tile([1, QH], FP, tag="rd",
                                name=f"rd_{b}_{h}_{half}")
                nc.vector.reciprocal_approx_fast(rd[:], rdsrc[:])
                rdb = tailp.tile([F, QH], FP, tag="rdb",
                                 name=f"rdb_{b}_{h}_{half}")
                nc.gpsimd.partition_broadcast(rdb[:], rd[:])
                nc.vector.tensor_mul(st["outT"][h * F:(h + 1) * F, hs2],
                                     st["ysb"][h][0:F, hs2], rdb[:])

            def c_parts(b):
                st = state[b]

                def c1(sb, scalar_copy=False):
                    ss = slice(sb * 128, (sb + 1) * 128)
                    ot = opool.tile([128, D], BF, tag="osb",
                                    name=f"ot_{b}_{sb}")
                    for n in range(D // 512):
                        pool = ps_a if n == 0 else ps_s
                        psp = pool.tile([128, 512], FP, tag="a" if n == 0
                                        else "s", name=f"psp_{b}_{sb}_{n}")
                        nc.tensor.matmul(
                            psp[:], st["outT"][:, ss],
                            wp_t[:, n * 512:(n + 1) * 512],
                            start=True, stop=True,
                        )
                        if scalar_copy and n == 0:
                            nc.scalar.copy(ot[:, n * 512:(n + 1) * 512],
                                           psp[:])
                        else:
                            nc.vector.tensor_copy(
                                ot[:, n * 512:(n + 1) * 512], psp[:])
                    nc.sync.dma_start(outp.ap()[b, ss, :], ot[:])

                return [
                    lambda sb=sb: c1(sb, scalar_copy=(b == B - 1 and sb >= 8))
                    for sb in range(S // 128)
                ]

            # ---- software-pipelined emission ----
            # Chunks of batch b interleave with phase A of b+1 and the
            # output projections whose outT halves are complete.
            cl = {}
            for part in a_parts(0):
                part()
            for b in range(B):
                cl[b] = c_parts(b)
                ap = a_parts(b + 1) if b + 1 < B else []
                cprev = cl[b - 1][8:16] if b >= 1 else []
                ccur = cl[b][0:8]
                plan = [
                    (0, ap[0:5] + cprev[0:4], []),
                    (1, ap[5:10] + cprev[4:8],
                     [lambda: tail(b, 0, 0), lambda: tail(b, 1, 0)]),
                    (2, ap[10:16] + ccur[0:4], []),
                    (3, ap[16:21] + ccur[4:8],
                     [lambda: tail(b, 0, 1), lambda: tail(b, 1, 1)]),
                ]
                for c, fillers, endf in plan:
                    b_chunk(b, c, fillers, endf)
            for part in cl[B - 1][8:16]:
                part()

    nc.compile()
    return nc

_NC_CACHE = []


def _get_nc():
    if not _NC_CACHE:
        _NC_CACHE.append(_build_nc())
    return _NC_CACHE[0]


def make_in_maps(sin, Wx, bx, Wp):
    """Host-side sharding: per-core input dicts."""
    # [B, D, S] -> contiguous tiles [B, KT, S//QH, 128, QH] so each sin DMA
    # is one 256KB contiguous read
    sinT = np.transpose(sin, (0, 2, 1)).reshape(B, KT, 128, S // QH, QH)
    sinT = np.ascontiguousarray(
        np.transpose(sinT, (0, 1, 3, 2, 4)).astype(NP_BF)
    )
    in_maps = []
    for c in range(NCORES):
        hs = slice(c * HPC, (c + 1) * HPC)
        # [D, FL] stacked head projections -> [128, KT*FL] k-tile-major
        wxl = np.concatenate([Wx[h] for h in range(c * HPC, (c + 1) * HPC)],
                             axis=1)
        wxl = np.ascontiguousarray(
            wxl.reshape(KT, 128, FL).transpose(1, 0, 2).reshape(128, KT * FL)
        ).astype(NP_BF)
        bxl = np.ascontiguousarray(bx[hs].reshape(FL, 1))
        wpl = np.ascontiguousarray(Wp[c * FL:(c + 1) * FL, :]).astype(NP_BF)
        in_maps.append({"sinT": sinT, "wxl": wxl, "bxl": bxl, "wpl": wpl})
    return in_maps


def make_runner(sin, Wx, bx, Wp):
    """Build a repeat-callable single-execution runner with device-resident
    inputs.

    Outputs are fed back as the donated output buffers, so each call is
    dispatch + device execution only (no host transfers). Returns
    (run_once, block) where run_once() dispatches one execution
    asynchronously and block() waits for all dispatched work.
    """
    import jax
    from concourse import bass2jax as b2j
    from concourse import mybir as _mb

    nc = _get_nc()
    b2j.install_neuronx_cc_hook()
    in_maps = make_in_maps(
        np.asarray(sin, np.float32), np.asarray(Wx, np.float32),
        np.asarray(bx, np.float32), np.asarray(Wp, np.float32),
    )

    in_names, out_names, out_avals, zero_outs = [], [], [], []
    for alloc in nc.m.functions[0].allocations:
        if not isinstance(alloc, _mb.MemoryLocationSet):
            continue
        name = alloc.memorylocations[0].name
        if alloc.kind == "ExternalInput":
            if nc.partition_id_tensor is None or name != nc.partition_id_tensor.name:
                in_names.append(name)
        elif alloc.kind == "ExternalOutput":
            out_names.append(name)
            shape = tuple(alloc.tensor_shape)
            dtype = _mb.dt.np(alloc.dtype)
            out_avals.append(jax.core.ShapedArray(shape, dtype))
            zero_outs.append(np.zeros(shape, dtype))
    n_params = len(in_names)
    n_outs = len(out_avals)
    all_names = in_names + out_names
    donate = tuple(range(n_params, n_params + n_outs))

    pid_name = nc.partition_id_tensor.name if nc.partition_id_tensor else None
    body_names = all_names + ([pid_name] if pid_name else [])

    def _exec_once(ins_, outs_):
        operands = list(ins_) + list(outs_)
        if pid_name:
            operands.append(b2j.partition_id_tensor())
        outs = b2j._bass_exec_p.bind(
            *operands,
            out_avals=tuple(out_avals),
            in_names=tuple(body_names),
            out_names=tuple(out_names),
            lowering_input_output_aliases=(),
            sim_require_finite=True,
            sim_require_nnan=True,
            nc=nc,
        )
        return tuple(outs)

    def _body(*args):
        return _exec_once(args[:n_params], args[n_params:])

    devices = jax.devices()[:NCORES]
    mesh = b2j.Mesh(np.asarray(devices), ("core",))
    in_specs = (b2j.PartitionSpec("core"),) * (n_params + n_outs)
    out_specs = (b2j.PartitionSpec("core"),) * n_outs
    sharded = jax.jit(
        b2j.shard_map(_body, mesh=mesh, in_specs=in_specs,
                      out_specs=out_specs, check_rep=False),
        donate_argnums=donate, keep_unused=True,
    )
    sharding = jax.sharding.NamedSharding(mesh, b2j.PartitionSpec("core"))
    concat_in = [
        jax.device_put(
            np.concatenate([np.asarray(in_maps[c][nm]) for c in range(NCORES)],
                           axis=0),
            sharding,
        )
        for nm in in_names
    ]
    outs = [
        jax.device_put(np.zeros((NCORES * z.shape[0], *z.shape[1:]), z.dtype),
                       sharding)
        for z in zero_outs
    ]
    jax.block_until_ready(concat_in)

    state = {"outs": outs}

    def run_once():
        state["outs"] = sharded(*concat_in, *state["outs"])

    def block():
        jax.block_until_ready(state["outs"])

    return run_once, block


def benchmark(sin, Wx, bx, Wp, iters=10, loop_n=1, runner=None):
    """Timed loop of the compiled executable; returns per-exec ns."""
    import time as _time

    run_once, block = runner or make_runner(sin, Wx, bx, Wp)
    times = []
    for _ in range(iters):
        t0 = _time.perf_counter()
        for _k in range(loop_n):
            run_once()
        block()
        times.append((_time.perf_counter() - t0) * 1e9 / loop_n)
    return times


def kernel(sin, Wx, bx, Wp, bp, _trace=False):
    sin = np.asarray(sin, dtype=np.float32)
    Wx = np.asarray(Wx, dtype=np.float32)
    bx = np.asarray(bx, dtype=np.float32)
    Wp = np.asarray(Wp, dtype=np.float32)
    bp = np.asarray(bp, dtype=np.float32)

    nc = _get_nc()
    in_maps = make_in_maps(sin, Wx, bx, Wp)
    res = run_bass_kernel_spmd(nc, in_maps, list(range(NCORES)), trace=_trace)
    out = np.sum(np.stack([np.asarray(r["outp"], np.float32)
                       for r in res.results]), axis=0) + bp
    if _trace:
        kernel.last_results = res
    return out.astype(np.float32)


# revision 14
# speedup vs baseline: 1.0971x; 1.0520x over previous
"""Multi-head attention (cosine-similarity scores, q=k=v) on 8 trn2 cores.

Reference computation (per head h, batch b):
    h_bh = sin_b @ Wx_h + bx_h                       # [S, F]
    C    = (h_bh h_bh^T) / (|h_s||h_t|)              # cosine scores, symmetric
    P    = softmax(C, axis=-1)                       # no max-shift needed: |C|<=1
    out_bh = P @ h_bh                                # [S, F]
    out_b  = concat_h(out_bh) @ Wp + bp              # [S, D]

Sharding: tensor-parallel over heads. Each core owns HPC=2 heads, computes the
partial output projection for its heads over the full batch, and the host sums
the 8 partials (+bp).

Layout: all score/value matmuls run column-wise [t-partition, s-free]; C's
symmetry makes exp(C) stored column-wise exactly the E[s,t] operand for
Y^T = h^T E.  The two heads live on partitions 0-63 / 64-127, so their K=64
score matmuls run CONCURRENTLY in the PE array (row-group tiling) and one
[128,1024] exp covers both heads (ACT is the critical engine; its per-call
overhead is amortized at the max width 4 PSUM banks allow).  The softmax
denominator rides as a ones-column in the Y stationary (psum row 64).  rsqrt
of the norms is exp(-0.5*ln(x)) so the whole kernel uses ONE ACT table set
(natural_log_exp_and_others) - no table swaps.  Norm reciprocals are
partition-broadcast with tiny K=1 matmuls (gpsimd broadcast can only read
partition 0; PE does it for free in the slack).
"""
import numpy as np

import concourse.bacc as bacc
import concourse.tile as tile
import concourse.mybir as mybir
from concourse import bass_isa, masks
from concourse.bass_utils import run_bass_kernel_spmd

B, S, D, H, F = 4, 2048, 1024, 16, 64
NCORES = 8
HPC = H // NCORES          # 2 heads per core
FL = HPC * F               # 128 local feature columns
SCH = 512                  # s-chunk width (one psy bank)
NCH = S // SCH             # 4 chunks per batch
KT = D // 128              # 8 contraction tiles for the input projection
NT0 = S // 128             # 16 t-blocks
QH = 1024                  # sin DMA block width
AUGW = NT0 * (F + 1)       # 1040 aug columns per head

FP = mybir.dt.float32
BF = mybir.dt.bfloat16
NP_BF = mybir.dt.np(mybir.dt.bfloat16)
AF = mybir.ActivationFunctionType


def _patch_act_tables():
    """Force Ln and Exp to resolve to the combined
    natural_log_exp_and_others set: with the default tables the chooser
    puts them in different sets and reloads ACT tables between every
    ln/exp pair (1.3us each, 16 loads per kernel).  Emptying the two
    narrower sets (order and ids preserved) leaves the combined set as
    the only candidate, so the whole kernel runs on ONE table load."""
    import concourse.hw_specs as hw_specs

    if getattr(bacc, "_act_tables_patched", False):
        return
    orig = hw_specs.get_activation_tables

    def patched(arch):
        t = dict(orig(arch))
        for k in ("exp_and_others", "natural_log"):
            if k in t:
                t[k] = set()
        return t

    bacc.get_activation_tables = patched
    bacc._act_tables_patched = True


def _build_nc():
    _patch_act_tables()
    nc = bacc.Bacc("TRN2", target_bir_lowering=False, debug=False)

    sinT = nc.dram_tensor("sinT", [B, KT, 128, S], BF,
                          kind="ExternalInput")
    wxl = nc.dram_tensor("wxl", [128, KT * FL], BF, kind="ExternalInput")
    bxl = nc.dram_tensor("bxl", [FL, 1], FP, kind="ExternalInput")
    wpl = nc.dram_tensor("wpl", [FL, D], BF, kind="ExternalInput")
    outp = nc.dram_tensor("outp", [B, S, D], BF, kind="ExternalOutput")

    with tile.TileContext(nc) as tc:
        with (
            tc.tile_pool(name="const", bufs=1) as constp,
            tc.tile_pool(name="wpool", bufs=1) as wpool,
            tc.tile_pool(name="sin", bufs=8) as sinp,
            tc.tile_pool(name="pa", bufs=1) as pa,
            tc.tile_pool(name="pb", bufs=2) as pb,
            tc.tile_pool(name="epool", bufs=4) as epool,
            tc.tile_pool(name="tailp", bufs=2) as tailp,
            tc.tile_pool(name="opool", bufs=3) as opool,
            # 8 PSUM banks:
            #   ps_c  2 x [128,1024] = 4 banks (paired score blocks)
            #   ps_y  2 x [65,512]   = 2 banks (per-head Y accumulators)
            #   ps_a  1 x [128,512]  = 1 bank  (proj accum / psn1 / rnb / outproj)
            #   ps_s  1 x [128,512]  = 1 bank  (psn0 / rnb / transposes / outproj)
            tc.tile_pool(name="ps_c", bufs=2, space="PSUM") as ps_c,
            tc.tile_pool(name="ps_y", bufs=1, space="PSUM") as ps_y,
            tc.tile_pool(name="ps_a", bufs=1, space="PSUM") as ps_a,
            tc.tile_pool(name="ps_s", bufs=1, space="PSUM") as ps_s,
        ):
            # ---- constants / weights ----
            ident = constp.tile([128, 128], FP, tag="ident")
            masks.make_identity(nc, ident[:])

            # ones2[:, h] = 1 on partitions h*64..h*64+63 (norm reductions)
            ones2_f = constp.tile([128, 2], FP, tag="ones2f")
            nc.vector.memset(ones2_f[:], 0.0)
            nc.vector.memset(ones2_f[0:64, 0:1], 1.0)
            nc.vector.memset(ones2_f[64:128, 1:2], 1.0)
            ones2 = constp.tile([128, 2], BF, tag="ones2")
            nc.vector.tensor_copy(ones2[:], ones2_f[:])

            # all-ones [128,64] bf16: K=1 broadcast-matmul stationary
            onesb_f = constp.tile([128, 64], FP, tag="onesbf")
            nc.vector.memset(onesb_f[:], 1.0)
            onesb = constp.tile([128, 64], BF, tag="onesb")
            nc.vector.tensor_copy(onesb[:], onesb_f[:])

            ones16_f = constp.tile([128, NT0], FP, tag="ones16f")
            nc.vector.memset(ones16_f[:], 1.0)

            wx_t = wpool.tile([128, KT * FL], BF, tag="wx")
            nc.sync.dma_start(wx_t[:], wxl.ap())
            bx_t = wpool.tile([FL, 1], FP, tag="bx")
            nc.sync.dma_start(bx_t[:], bxl.ap())
            wp_t = wpool.tile([FL, D], BF, tag="wp")
            nc.sync.dma_start(wp_t[:], wpl.ap())

            state = {}

            def a_parts(b):
                """Emitters for phase A of batch b: projection, norms,
                normalization, aug build.  Returned as a list of parts to be
                interleaved into the previous batch's attention chunks."""
                st = {}
                state[b] = st
                sints = {}

                def mk():
                    st["hT"] = pa.tile([128, S], FP, tag="hT", name=f"hT_{b}")
                    st["sqt"] = pa.tile([128, S], BF, tag="sq", name=f"sq_{b}")
                    st["hTn"] = pb.tile([128, S], BF, tag="hTn",
                                        name=f"hTn_{b}")
                    st["aug"] = pb.tile([128, HPC * AUGW], BF, tag="aug",
                                        name=f"aug_{b}")
                    st["outT"] = pb.tile([128, S], BF, tag="outT",
                                         name=f"outT_{b}")
                    st["ysb"] = [
                        pb.tile([F + 1, S], FP, tag=f"ysb{h}",
                                name=f"ysb{h}_{b}")
                        for h in range(HPC)
                    ]
                    st["rn"] = [
                        pa.tile([128, SCH], BF, tag=f"rn{h}",
                                name=f"rn{h}_{b}")
                        for h in range(HPC)
                    ]

                def a1d():
                    # one 512KB DMA per k-tile (sync-engine dispatch is
                    # ~650ns per DMA; fewer, bigger transfers)
                    mk()
                    for k in range(KT):
                        sint = sinp.tile([128, S], BF, tag="sin",
                                         name=f"sin_{b}_{k}")
                        nc.sync.dma_start(sint[:], sinT.ap()[b, k])
                        sints[k] = sint

                def a1q(half, q):
                    # one half (4 k-tiles) of a 512-wide projection accum
                    cs = slice(half * SCH, (half + 1) * SCH)
                    if q == 0:
                        st["pshT"] = ps_a.tile([128, SCH], FP, tag="a",
                                               name=f"pshT_{b}_{half}")
                    pshT = st["pshT"]
                    for k in range(q * KT // 2, (q + 1) * KT // 2):
                        nc.tensor.matmul(
                            pshT[:], wx_t[:, k * FL:(k + 1) * FL],
                            sints[k][:, cs],
                            start=(k == 0), stop=(k == KT - 1),
                        )
                    if q == 1:
                        nc.vector.tensor_scalar_add(st["hT"][:, cs], pshT[:],
                                                    bx_t[:])
                        nc.vector.tensor_mul(st["sqt"][:, cs], st["hT"][:, cs],
                                             st["hT"][:, cs])

                def norms(h):
                    # norm^2 rows for head h -> psum partitions {0,32,64,96}
                    # (chunk c at row c*32), then rnorm = exp(-0.5*ln(x)):
                    # same ACT table set as the attention exp, no table swaps.
                    # Compact: the psum slot is created and consumed within
                    # this one part (the 1-buf pools rotate with outproj).
                    pool = ps_s if h == 0 else ps_a
                    psn = pool.tile([128, SCH], FP, tag="s" if h == 0
                                    else "a", name=f"psn{h}_{b}")
                    for c in range(NCH):
                        cs = slice(c * SCH, (c + 1) * SCH)
                        nc.tensor.matmul(psn[c * 32:c * 32 + 1, :],
                                         ones2[:, h:h + 1], st["sqt"][:, cs],
                                         start=True, stop=True,
                                         tile_position=(0, c * 32))
                    lnt = pa.tile([128, SCH], FP, tag="lnt",
                                  name=f"lnt_{b}_{h}")
                    nc.scalar.activation(lnt[:], psn[:], AF.Ln)
                    nc.scalar.activation(st["rn"][h][:], lnt[:], AF.Exp,
                                         scale=-0.5)

                def rnb(c):
                    # broadcast rnorm rows to [128,512] via K=1 matmuls,
                    # then one mul makes the normalized hTn chunk
                    pool = ps_s if c % 2 == 0 else ps_a
                    cs = slice(c * SCH, (c + 1) * SCH)
                    prn = pool.tile([128, SCH], FP, tag="s" if c % 2 == 0
                                    else "a", name=f"prn_{b}_{c}")
                    r = c * 32
                    nc.tensor.matmul(prn[0:64, :], onesb[r:r + 1, :],
                                     st["rn"][0][r:r + 1, :],
                                     start=True, stop=True,
                                     tile_position=(r, 0))
                    nc.tensor.matmul(prn[64:128, :], onesb[r:r + 1, :],
                                     st["rn"][1][r:r + 1, :],
                                     start=True, stop=True,
                                     tile_position=(r, 64))
                    nc.vector.tensor_mul(st["hTn"][:, cs], st["hT"][:, cs],
                                         prn[:])

                def a3(q):
                    # 4 transposes: hT [f,t] -> aug [t,f] for both heads
                    for t0 in range(q * 4, q * 4 + 4):
                        pool = ps_s if t0 % 2 == 0 else ps_a
                        pst = pool.tile([128, 128], FP, tag="s" if t0 % 2 == 0
                                        else "a", name=f"pst_{b}_{t0}")
                        nc.tensor.transpose(
                            pst[:], st["hT"][:, t0 * 128:(t0 + 1) * 128],
                            ident[:]
                        )
                        dst = st["aug"][:].rearrange(
                            "p (h t f) -> p h t f", h=HPC, f=F + 1
                        )[:, :, t0, 0:F]
                        src = pst[:].rearrange("p (h f) -> p h f", h=HPC)
                        nc.vector.tensor_copy(dst, src)

                def a4():
                    for h in range(HPC):
                        ones_col = st["aug"][:].rearrange(
                            "p (h t f) -> p h t f", h=HPC, f=F + 1
                        )[:, h, :, F:F + 1]
                        nc.vector.tensor_copy(ones_col, ones16_f[:])

                return (
                    [a1d,
                     lambda: a1q(0, 0), lambda: a1q(0, 1),
                     lambda: a1q(1, 0), lambda: a1q(1, 1),
                     lambda: a1q(2, 0), lambda: a1q(2, 1),
                     lambda: a1q(3, 0), lambda: a1q(3, 1),
                     lambda: norms(0), lambda: norms(1)]
                    + [lambda c=c: rnb(c) for c in range(NCH)]
                    + [lambda q=q: a3(q) for q in range(4)]
                    + [a4]
                )

            def b_chunk(b, c, fillers=(), end_fillers=()):
                """One 512-wide s-chunk: both heads, all 16 t-blocks."""
                st = state[b]
                cs = slice(c * SCH, (c + 1) * SCH)
                fillers = list(fillers)
                psy = [
                    ps_y.tile([F + 1, SCH], FP, tag=f"y{h}",
                              name=f"psy{h}_{b}_{c}")
                    for h in range(HPC)
                ]
                aug4 = st["aug"][:].rearrange("p (h t f) -> p h t f",
                                              h=HPC, f=F + 1)
                for t0 in range(NT0):
                    ts0 = slice(t0 * 128, (t0 + 1) * 128)
                    psc = ps_c.tile([128, 2 * SCH], FP, tag="c",
                                    name=f"psc_{b}_{c}_{t0}")
                    for h in range(HPC):
                        hs = slice(h * F, (h + 1) * F)
                        nc.tensor.matmul(
                            psc[:, h * SCH:(h + 1) * SCH],
                            st["hTn"][hs, ts0], st["hTn"][hs, cs],
                            start=True, stop=True,
                        )
                    et = epool.tile([128, 2 * SCH], BF, tag="E",
                                    name=f"E_{b}_{c}_{t0}")
                    nc.scalar.activation(et[:], psc[:], AF.Exp)
                    for h in range(HPC):
                        nc.tensor.matmul(
                            psy[h][:], aug4[:, h, t0, :],
                            et[:, h * SCH:(h + 1) * SCH],
                            start=(t0 == 0), stop=(t0 == NT0 - 1),
                        )
                    if fillers:
                        fillers.pop(0)()
                # release the psy banks quickly into the per-head accums
                for h in range(HPC):
                    nc.vector.tensor_copy(st["ysb"][h][:, cs], psy[h][:])
                for part in fillers:
                    part()
                for part in end_fillers:
                    part()

            def tail(b, h, half):
                """Divide accumulated Y by the softmax denominators."""
                st = state[b]
                hs2 = slice(half * QH, (half + 1) * QH)
                rdsrc = tailp.tile([1, QH], FP, tag="rdsrc",
                                   name=f"rdsrc_{b}_{h}_{half}")
                nc.vector.tensor_copy(rdsrc[:], st["ysb"][h][F:F + 1, hs2])
                rd = tailp.tile([1, QH], FP, tag="rd",
                                name=f"rd_{b}_{h}_{half}")
                nc.vector.reciprocal_approx_fast(rd[:], rdsrc[:])
                rdb = tailp.tile([F, QH], FP, tag="rdb",
                                 name=f"rdb_{b}_{h}_{half}")
                nc.gpsimd.partition_broadcast(rdb[:], rd[:])
                nc.vector.tensor_mul(st["outT"][h * F:(h + 1) * F, hs2],
                                     st["ysb"][h][0:F, hs2], rdb[:])

            def c_parts(b):
                st = state[b]

                def c1(sb, scalar_copy=False):
                    ss = slice(sb * 128, (sb + 1) * 128)
                    ot = opool.tile([128, D], BF, tag="osb",
                                    name=f"ot_{b}_{sb}")
                    for n in range(D // 512):
                        pool = ps_a if n == 0 else ps_s
                        psp = pool.tile([128, 512], FP, tag="a" if n == 0
                                        else "s", name=f"psp_{b}_{sb}_{n}")
                        nc.tensor.matmul(
                            psp[:], st["outT"][:, ss],
                            wp_t[:, n * 512:(n + 1) * 512],
                            start=True, stop=True,
                        )
                        if scalar_copy and n == 0:
                            nc.scalar.copy(ot[:, n * 512:(n + 1) * 512],
                                           psp[:])
                        else:
                            nc.vector.tensor_copy(
                                ot[:, n * 512:(n + 1) * 512], psp[:])
                    nc.sync.dma_start(outp.ap()[b, ss, :], ot[:])

                return [
                    lambda sb=sb: c1(sb, scalar_copy=(b == B - 1 and sb >= 8))
                    for sb in range(S // 128)
                ]

            # ---- software-pipelined emission ----
            # Chunks of batch b interleave with phase A of b+1 and the
            # output projections whose outT halves are complete.
            cl = {}
            for part in a_parts(0):
                part()
            for b in range(B):
                cl[b] = c_parts(b)
                ap = a_parts(b + 1) if b + 1 < B else []
                cprev = cl[b - 1][8:16] if b >= 1 else []
                ccur = cl[b][0:8]
                plan = [
                    (0, ap[0:6] + cprev[0:4], []),
                    (1, ap[6:11] + cprev[4:8],
                     [lambda: tail(b, 0, 0), lambda: tail(b, 1, 0)]),
                    (2, ap[11:20], []),
                    (3, ccur[0:8],
                     [lambda: tail(b, 0, 1), lambda: tail(b, 1, 1)]),
                ]
                for c, fillers, endf in plan:
                    b_chunk(b, c, fillers, endf)
            for part in cl[B - 1][8:16]:
                part()

    nc.compile()
    return nc

_NC_CACHE = []


def _get_nc():
    if not _NC_CACHE:
        _NC_CACHE.append(_build_nc())
    return _NC_CACHE[0]


def make_in_maps(sin, Wx, bx, Wp):
    """Host-side sharding: per-core input dicts."""
    # [B, D, S] -> contiguous slabs [B, KT, 128, S]: each sin DMA is one
    # contiguous 512KB read
    sinT = np.ascontiguousarray(
        np.transpose(sin, (0, 2, 1)).reshape(B, KT, 128, S).astype(NP_BF)
    )
    in_maps = []
    for c in range(NCORES):
        hs = slice(c * HPC, (c + 1) * HPC)
        # [D, FL] stacked head projections -> [128, KT*FL] k-tile-major
        wxl = np.concatenate([Wx[h] for h in range(c * HPC, (c + 1) * HPC)],
                             axis=1)
        wxl = np.ascontiguousarray(
            wxl.reshape(KT, 128, FL).transpose(1, 0, 2).reshape(128, KT * FL)
        ).astype(NP_BF)
        bxl = np.ascontiguousarray(bx[hs].reshape(FL, 1))
        wpl = np.ascontiguousarray(Wp[c * FL:(c + 1) * FL, :]).astype(NP_BF)
        in_maps.append({"sinT": sinT, "wxl": wxl, "bxl": bxl, "wpl": wpl})
    return in_maps


def make_runner(sin, Wx, bx, Wp):
    """Build a repeat-callable single-execution runner with device-resident
    inputs.

    Outputs are fed back as the donated output buffers, so each call is
    dispatch + device execution only (no host transfers). Returns
    (run_once, block) where run_once() dispatches one execution
    asynchronously and block() waits for all dispatched work.
    """
    import jax
    from concourse import bass2jax as b2j
    from concourse import mybir as _mb

    nc = _get_nc()
    b2j.install_neuronx_cc_hook()
    in_maps = make_in_maps(
        np.asarray(sin, np.float32), np.asarray(Wx, np.float32),
        np.asarray(bx, np.float32), np.asarray(Wp, np.float32),
    )

    in_names, out_names, out_avals, zero_outs = [], [], [], []
    for alloc in nc.m.functions[0].allocations:
        if not isinstance(alloc, _mb.MemoryLocationSet):
            continue
        name = alloc.memorylocations[0].name
        if alloc.kind == "ExternalInput":
            if nc.partition_id_tensor is None or name != nc.partition_id_tensor.name:
                in_names.append(name)
        elif alloc.kind == "ExternalOutput":
            out_names.append(name)
            shape = tuple(alloc.tensor_shape)
            dtype = _mb.dt.np(alloc.dtype)
            out_avals.append(jax.core.ShapedArray(shape, dtype))
            zero_outs.append(np.zeros(shape, dtype))
    n_params = len(in_names)
    n_outs = len(out_avals)
    all_names = in_names + out_names
    donate = tuple(range(n_params, n_params + n_outs))

    pid_name = nc.partition_id_tensor.name if nc.partition_id_tensor else None
    body_names = all_names + ([pid_name] if pid_name else [])

    def _exec_once(ins_, outs_):
        operands = list(ins_) + list(outs_)
        if pid_name:
            operands.append(b2j.partition_id_tensor())
        outs = b2j._bass_exec_p.bind(
            *operands,
            out_avals=tuple(out_avals),
            in_names=tuple(body_names),
            out_names=tuple(out_names),
            lowering_input_output_aliases=(),
            sim_require_finite=True,
            sim_require_nnan=True,
            nc=nc,
        )
        return tuple(outs)

    def _body(*args):
        return _exec_once(args[:n_params], args[n_params:])

    devices = jax.devices()[:NCORES]
    mesh = b2j.Mesh(np.asarray(devices), ("core",))
    in_specs = (b2j.PartitionSpec("core"),) * (n_params + n_outs)
    out_specs = (b2j.PartitionSpec("core"),) * n_outs
    sharded = jax.jit(
        b2j.shard_map(_body, mesh=mesh, in_specs=in_specs,
                      out_specs=out_specs, check_rep=False),
        donate_argnums=donate, keep_unused=True,
    )
    sharding = jax.sharding.NamedSharding(mesh, b2j.PartitionSpec("core"))
    concat_in = [
        jax.device_put(
            np.concatenate([np.asarray(in_maps[c][nm]) for c in range(NCORES)],
                           axis=0),
            sharding,
        )
        for nm in in_names
    ]
    outs = [
        jax.device_put(np.zeros((NCORES * z.shape[0], *z.shape[1:]), z.dtype),
                       sharding)
        for z in zero_outs
    ]
    jax.block_until_ready(concat_in)

    state = {"outs": outs}

    def run_once():
        state["outs"] = sharded(*concat_in, *state["outs"])

    def block():
        jax.block_until_ready(state["outs"])

    return run_once, block


def benchmark(sin, Wx, bx, Wp, iters=10, loop_n=1, runner=None):
    """Timed loop of the compiled executable; returns per-exec ns."""
    import time as _time

    run_once, block = runner or make_runner(sin, Wx, bx, Wp)
    times = []
    for _ in range(iters):
        t0 = _time.perf_counter()
        for _k in range(loop_n):
            run_once()
        block()
        times.append((_time.perf_counter() - t0) * 1e9 / loop_n)
    return times


def kernel(sin, Wx, bx, Wp, bp, _trace=False):
    sin = np.asarray(sin, dtype=np.float32)
    Wx = np.asarray(Wx, dtype=np.float32)
    bx = np.asarray(bx, dtype=np.float32)
    Wp = np.asarray(Wp, dtype=np.float32)
    bp = np.asarray(bp, dtype=np.float32)

    nc = _get_nc()
    in_maps = make_in_maps(sin, Wx, bx, Wp)
    res = run_bass_kernel_spmd(nc, in_maps, list(range(NCORES)), trace=_trace)
    out = np.sum(np.stack([np.asarray(r["outp"], np.float32)
                       for r in res.results]), axis=0) + bp
    if _trace:
        kernel.last_results = res
    return out.astype(np.float32)
